# revision 1
# baseline (speedup 1.0000x reference)
"""Trainium2 Bass kernel for nn_LinearAttention (RoPE(Q) @ RoPE(Q)^T @ V).

Key algebraic insight: there is no softmax, so
    out = (QR @ QR^T) @ V  ==  QR @ (QR^T @ V)
which replaces the [T,T] score matrix with a [d,d] (64x64) intermediate:
~32x fewer FLOPs. Sharding: 16 heads / 8 cores = 2 heads per core, no
cross-core communication.

Layout: the t-axis is permuted into 16 chunks (t = p*16 + c, p = SBUF
partition). Valid because the contraction sums over all t and the second
matmul is row-local in t; the host packs/unpacks with the same
permutation. The two heads ride in the two 64-partition "lanes" of the
128x128 PE array (head h occupies d-rows/columns 64h:64h+64):

  1. RoPE on Q (DVE + GpSimd share the elementwise work; Q arrives
     pre-split into rotate-half halves so every op is 2D-contiguous).
  2. S2 = sum_c [qr_c(h0)|qr_c(h1)]^T @ [v_c(h0)|v_c(h1)]
     (16 accumulating matmuls N=128; diagonal 64x64 blocks are S_h).
  3. QRT_c = PE-transpose of [qr_c(h0)|qr_c(h1)]  -> both heads' lanes.
  4. outT blocks = blockdiag(S_h0,S_h1)^T @ QRT (4 matmuls N=512; the
     zero off-diagonal blocks kill the cross-head terms).
  5. Four DMAs stream outT out as blocks complete; the host undoes the
     transpose during unsharding.

Perf notes baked in: matmul operands are float32r end-to-end (fp32
streams the moving operand at 2 cycles/column, fp32r at 1); a burst of
dependency-free garbage transposes keeps the PE busy from the preamble
on, so the HAM clock-gate reaches 2.4 GHz before the real matmul
stream; all elementwise ops use fully contiguous 2D access patterns
(multi-dim strided APs hit a DVE slow path ~3x); the Tile kernel-tail
drain+barrier is replaced with a slim per-engine-drain + sem-only
barrier (the default EVSEM butterfly costs ~8 us).

The compiler build allows only ONE sync-wait per engine instruction and
Tile's wait elision is per-engine, so: input DMAs land in SBUF-native
layout (host pre-packs), tiny per-engine "absorber" ops observe each DMA
semaphore once, and cross-engine produced tiles are grouped per consumer
engine. A post-pass splits any remaining multi-wait instruction into
single-wait NoOps.
"""

from contextlib import ExitStack

import numpy as np

import concourse.bass as bass
import concourse.mybir as mybir
import concourse.tile as tile
from concourse.bass_utils import run_bass_kernel_spmd
from concourse.vector_clock import ScopedClock

H, T, D = 16, 2048, 64
N_CORES = 8
HPC = H // N_CORES  # heads per core
P = 128
NT = T // P  # 16 t-chunks per head
HD = D // 2
NTAB = 2 * NT * HPC * HD + P  # cosE | sinE ([2, HPC, 8, HD] each) | idt
F32 = mybir.dt.float32
F32R = mybir.dt.float32r
BF16 = mybir.dt.bfloat16
N_WARM = 22  # dep-free garbage transposes to spin HAM up to 2.4 GHz early


def _rope_tables():
    inv_freq = 1.0 / (10000.0 ** (np.arange(0, D, 2, dtype=np.float32) / D))
    t = np.arange(T, dtype=np.float32)
    freqs = np.outer(t, inv_freq).astype(np.float32)  # [T, D/2]
    return np.cos(freqs).astype(np.float32), np.sin(freqs).astype(np.float32)


class _SlimTileContext(tile.TileContext):
    """TileContext whose kernel tail uses per-engine drains + a
    sequencer-level (sem-only) barrier instead of the full EVSEM
    butterfly. Semantics kept: SP's drain still waits on every live
    semaphore's final value (split into single-wait NoOps later), each
    engine's pipeline is drained before the semaphore range-clear, and a
    final sem-only barrier orders the clear before the NEFF ends."""

    def _drain_and_barrier(self, tick_clock, wait_clock):
        nc = self.nc
        drain_inst = nc.sync.drain()
        wait_clock.add_sem_waits(
            drain_inst.ins, ScopedClock({None: tick_clock.global_clock})
        )
        for eng in nc.engines.values():
            if eng.engine != mybir.EngineType.SP:
                eng.drain(fusable=False)
        nc.all_engine_barrier(sem_only=True)
        popped = nc._tile_sem_poison_stack.pop()
        assert popped is self._sem_poison
        nc.clear_and_free_semaphores(list(self.sems.allocated().values()))
        nc.all_engine_barrier(sem_only=True)


def _build_nc():
    nc = bass.Bass()
    TAB = nc.declare_dram_parameter("TAB", [P, NTAB], BF16, isOutput=False)
    # q pre-split into rotate-half halves: [head, half, chunk, k]
    QA = nc.declare_dram_parameter("QA", [P, HPC * 2 * 8 * HD], BF16, isOutput=False)
    QB = nc.declare_dram_parameter("QB", [P, HPC * 2 * 8 * HD], BF16, isOutput=False)
    VA = nc.declare_dram_parameter("VA", [P, 8 * HPC * D], BF16, isOutput=False)
    VB = nc.declare_dram_parameter("VB", [P, 8 * HPC * D], BF16, isOutput=False)
    OUT = nc.declare_dram_parameter("OUT", [P, T], BF16, isOutput=True)

    with _SlimTileContext(nc) as tc, ExitStack() as ctx:
        singles = ctx.enter_context(tc.tile_pool(name="singles", bufs=1))
        ps_s = ctx.enter_context(tc.tile_pool(name="ps_s", bufs=1, space="PSUM"))
        ps_tp = ctx.enter_context(tc.tile_pool(name="ps_tp", bufs=3, space="PSUM"))
        ps_o = ctx.enter_context(tc.tile_pool(name="ps_o", bufs=2, space="PSUM"))

        # Garbage-input PE warm-up: no data dependencies at all, so these
        # start right after the engine preamble and keep the PE busy
        # while the input DMAs land (HAM reaches 8/8 before real work).
        spam_src = singles.tile([P, P], F32)
        nc.gpsimd.memset(spam_src[:, 0:2], 0.0)
        for _ in range(N_WARM):
            warm = ps_tp.tile([P, P], F32, tag="tp")
            nc.tensor.transpose(warm, spam_src, spam_src)

        tab_sb = singles.tile([P, NTAB], BF16)
        # q layout: [range, half, head, chunk-in-range, k]
        q_sb = singles.tile([P, 2, 2, HPC, 8, HD], BF16)
        v_sb = singles.tile([P, NT, HPC, D], BF16)
        # Two HWDGE rings in parallel, ordered so RoPE's inputs land
        # first: SP streams QA then QB; ACT streams TAB then VA, VB.
        nc.sync.dma_start(
            out=q_sb[:, 0],
            in_=QA[:].rearrange("p (x h c k) -> p x h c k", x=2, h=HPC, c=8),
        )
        nc.scalar.dma_start(out=tab_sb, in_=TAB[:])
        nc.sync.dma_start(
            out=q_sb[:, 1],
            in_=QB[:].rearrange("p (x h c k) -> p x h c k", x=2, h=HPC, c=8),
        )
        nc.scalar.dma_start(
            out=v_sb[:, 0:8],
            in_=VA[:].rearrange("p (c h d) -> p c h d", c=8, h=HPC),
        )
        nc.scalar.dma_start(
            out=v_sb[:, 8:16],
            in_=VB[:].rearrange("p (c h d) -> p c h d", c=8, h=HPC),
        )

        idt = tab_sb[:, 2 * NT * HPC * HD :]

        qr_r = singles.tile([P, NT, HPC, 2, HD], BF16)
        qrtmp = singles.tile([P, 2, HPC * 8 * HD], BF16)
        tmp1 = singles.tile([P, HPC * 8 * HD], BF16)
        tmp2 = singles.tile([P, HPC * 8 * HD], BF16)
        qrt_sb = singles.tile([P, NT * P], BF16)
        s2d = singles.tile([P, P], BF16)
        outT_sb = singles.tile([P, T], BF16)
        scratch = singles.tile([P, 8], F32)

        # Absorbers + early table work (DVE and GpSimd observe the TAB
        # semaphore; the off-diagonal zeros of the phase-3 operand only
        # need the identity slab, so they run while waiting for Q/V).
        idt_r = singles.tile([P, P], BF16)
        nc.vector.tensor_copy(out=idt_r, in_=idt)
        nc.vector.tensor_scalar_mul(s2d[:D, D:], idt[:D, :D], 0.0)
        nc.vector.tensor_scalar_mul(s2d[D:, :D], idt[:D, :D], 0.0)

        s2_ps = ps_s.tile([P, P], F32)

        nexp = HPC * 8 * HD
        for half in range(2):
            r0 = half * 8
            cs = slice(r0, r0 + 8)
            cosr = tab_sb[:, half * nexp : (half + 1) * nexp]
            sinr = tab_sb[:, (2 + half) * nexp : (3 + half) * nexp]

            # RoPE over a whole chunk-range, both heads, per rotate-half
            # half: 6 ops of [128, 512], fully contiguous except the two
            # final writes (which scatter into the chunk-major qr tile in
            # matching (h, c, k) iteration order).
            #   qr_lo = q_lo*cos - q_hi*sin ; qr_hi = q_hi*cos + q_lo*sin
            qlo = q_sb[:, half, 0].rearrange("p h c k -> p (h c k)")
            qhi = q_sb[:, half, 1].rearrange("p h c k -> p (h c k)")
            # GpSimd takes the sin muls of range B so DVE reaches the
            # final combines sooner (range A keeps DVE-only to avoid
            # early port contention with the A-range combines).
            eng = nc.gpsimd if half == 1 else nc.vector
            eng.tensor_mul(tmp1, qhi, sinr)
            eng.tensor_mul(tmp2, qlo, sinr)
            nc.vector.tensor_mul(qrtmp[:, 0], qlo, cosr)
            nc.vector.tensor_mul(qrtmp[:, 1], qhi, cosr)
            qr_lo = qr_r[:, cs, :, 0, :].rearrange("p c h k -> p h c k")
            qr_hi = qr_r[:, cs, :, 1, :].rearrange("p c h k -> p h c k")
            shp = dict(h=HPC, c=8)
            nc.vector.tensor_sub(
                qr_lo, qrtmp[:, 0].rearrange("p (h c k) -> p h c k", **shp),
                tmp1.rearrange("p (h c k) -> p h c k", **shp),
            )
            nc.vector.tensor_add(
                qr_hi, qrtmp[:, 1].rearrange("p (h c k) -> p h c k", **shp),
                tmp2.rearrange("p (h c k) -> p h c k", **shp),
            )

            # PE observes this half's v-DMA semaphore once (result unused).
            warm2 = ps_tp.tile([P, P], BF16, tag="tp")
            nc.tensor.transpose(
                warm2, v_sb[:, r0].rearrange("p h d -> p (h d)"), idt_r
            )
            if half == 0:
                # Filler bridges the PE idle window between the warm-up
                # burst and the first real matmuls (RoPE-A still running
                # on DVE), so HAM's MID window never sees ~2 us of idle.
                for _ in range(6):
                    warm_f = ps_tp.tile([P, P], F32, tag="tp")
                    nc.tensor.transpose(warm_f, spam_src, spam_src)

            for c in range(r0, r0 + 8):
                # lhsT free order (h, half, k) = (h, d): the head lanes.
                qr2 = qr_r[:, c].rearrange("p h x k -> p (h x k)")
                v2 = v_sb[:, c].rearrange("p h d -> p (h d)")
                nc.tensor.matmul(
                    s2_ps, lhsT=qr2, rhs=v2, start=(c == 0), stop=(c == NT - 1)
                )
                # Transpose as a REGULAR matmul with the identity as the
                # moving operand (qr_c^T @ I): the moving-operand slot
                # requires a single free dimension, which qr2 (multi-dim
                # lhsT AP) cannot satisfy in transpose mode.
                tp = ps_tp.tile([P, P], F32, tag="tp")
                nc.tensor.matmul(tp, lhsT=qr2, rhs=idt_r, start=True, stop=True)
                # chunks 0-7 copy on ACT, 8-15 on DVE: splits the copy
                # load; phase 3's single DVE wait covers the DVE-copied
                # chunks transitively (s2d is copied later on DVE), and
                # the warm3 absorber covers the ACT-copied ones.
                if c < 8:
                    nc.scalar.copy(out=qrt_sb[:, c * P : (c + 1) * P], in_=tp)
                else:
                    nc.vector.tensor_copy(out=qrt_sb[:, c * P : (c + 1) * P], in_=tp)

        # Diagonal S_h blocks -> block-diagonal phase-3 operand.
        nc.vector.tensor_copy(out=s2d[:D, :D], in_=s2_ps[:D, :D])
        nc.vector.tensor_copy(out=s2d[D:, D:], in_=s2_ps[D:, D:])

        # PE observes the ACT semaphore once (after the last qrt copy).
        warm3 = ps_s.tile([8, P], BF16, tag="w3")
        nc.tensor.transpose(warm3, qrt_sb[:, 8 * P - 8 : 8 * P], idt_r)

        # outT blocks: blockdiag(S)^T @ QRT serves both heads at once.
        for i in range(4):
            o_ps = ps_o.tile([P, 512], F32, tag="o")
            blk = slice(i * 512, (i + 1) * 512)
            nc.tensor.matmul(
                o_ps, lhsT=s2d, rhs=qrt_sb[:, blk], start=True, stop=True
            )
            nc.vector.tensor_copy(out=outT_sb[:, blk], in_=o_ps)
            nc.sync.dma_start(out=OUT[:, blk], in_=outT_sb[:, blk])

    _split_multi_waits(nc)
    return nc


def _split_multi_waits(nc):
    """This compiler build rejects instructions carrying more than one
    sync-wait command. Tile's kernel-tail drain aggregates one wait per
    live semaphore, so split the extras into single-wait NoOps placed
    immediately before it on the same engine (sequential execution on the
    engine's queue preserves the barrier semantics)."""
    n = 0
    for f in nc.m.functions:
        for blk in f.blocks:
            new_insts = []
            for inst in blk.instructions:
                si = inst.sync_info
                waits = list(si.on_wait) if si else []
                if len(waits) > 1:
                    for w in waits[:-1]:
                        nop = mybir.InstNoOp(name=f"W-split-{n}", ins=[], outs=[])
                        n += 1
                        nop.engine = inst.engine
                        nop.sync_info = mybir.SyncInfo(on_wait=[w], on_update=[])
                        new_insts.append(nop)
                    inst.sync_info = mybir.SyncInfo(
                        on_wait=[waits[-1]], on_update=list(si.on_update)
                    )
                new_insts.append(inst)
            blk.instructions = new_insts


_NC_CACHE = None


def _get_nc():
    global _NC_CACHE
    if _NC_CACHE is None:
        _NC_CACHE = _build_nc()
    return _NC_CACHE


def _pack_inputs(Qs, Vs, cos32, sin32, idt):
    import ml_dtypes

    bf16 = ml_dtypes.bfloat16

    # [T, X] -> [P, NT, X] with t = p*NT + c
    def r(x):
        return x.reshape(P, NT, -1)

    # cosE[p, range, h, c, k] = cos32[t = p*16 + range*8 + c, k]
    ce = r(cos32).reshape(P, 2, 8, HD)  # [p, range, c, k]
    se = r(sin32).reshape(P, 2, 8, HD)
    cosE = np.repeat(ce[:, :, None, :, :], HPC, axis=2)  # [p, range, h, c, k]
    sinE = np.repeat(se[:, :, None, :, :], HPC, axis=2)
    tab = np.concatenate(
        [cosE.reshape(P, -1), sinE.reshape(P, -1), idt], axis=1
    ).astype(bf16)
    tab = np.ascontiguousarray(tab)

    in_maps = []
    for core in range(N_CORES):
        h0 = core * HPC
        # q[p, range, half, h, c, k], v[p, c, h, d]
        q = np.empty((P, 2, 2, HPC, 8, HD), np.float32)
        v = np.empty((P, NT, HPC, D), np.float32)
        for h in range(HPC):
            qh = r(Qs[h0 + h]).reshape(P, 2, 8, D)  # [p, range, c, d]
            q[:, :, 0, h] = qh[:, :, :, :HD]
            q[:, :, 1, h] = qh[:, :, :, HD:]
            v[:, :, h] = r(Vs[h0 + h])
        in_maps.append(
            {
                "TAB": tab,
                "QA": np.ascontiguousarray(q[:, 0].reshape(P, -1).astype(bf16)),
                "QB": np.ascontiguousarray(q[:, 1].reshape(P, -1).astype(bf16)),
                "VA": np.ascontiguousarray(v[:, 0:8].reshape(P, -1).astype(bf16)),
                "VB": np.ascontiguousarray(v[:, 8:16].reshape(P, -1).astype(bf16)),
            }
        )
    return in_maps


def _unpack_out(o):
    # o: [P, T] = outT; rows h*64+j, cols c-major: col = c*128 + f, t = f*16+c
    a = o.reshape(HPC, D, NT, P)  # [h, j, c, f]
    return a.transpose(0, 3, 2, 1).reshape(HPC, T, D)  # [h, t=f*16+c, j]


def run_inner(Q, K, V, trace=False):
    del K  # the module sets KR = QR; K is unused
    Qs = np.asarray(Q, dtype=np.float32)[0]  # [H, T, D]
    Vs = np.asarray(V, dtype=np.float32)[0]
    cos32, sin32 = _rope_tables()
    idt = np.eye(P, dtype=np.float32)
    nc = _get_nc()
    in_maps = _pack_inputs(Qs, Vs, cos32, sin32, idt)
    res = run_bass_kernel_spmd(nc, in_maps, list(range(N_CORES)), trace=trace)
    outs = [_unpack_out(np.asarray(res.results[i]["OUT"])) for i in range(N_CORES)]
    out = np.concatenate(outs, axis=0)[None]  # [1, H, T, D]
    return out.astype(np.float32), res


def kernel(Q, K, V):
    out, _ = run_inner(Q, K, V, trace=False)
    return out



# revision 5
# speedup vs baseline: 1.0956x; 1.0956x over previous
"""Trainium2 Bass kernel for nn_LinearAttention (RoPE(Q) @ RoPE(Q)^T @ V).

Algebraic core: no softmax, so out = (QR@QR^T)@V == QR@(QR^T@V) with a
64x64 per-head intermediate. 16 heads / 8 cores = 2 heads per core; the
two heads ride the two 64-wide lanes of the 128x128 PE array.

Layout: t = p*16 + (r*8 + c) (p = SBUF partition, r = range 0/1,
c = chunk-in-range); the host packs/unpacks with this permutation.

v2 schedule (vs the 31us baseline):
  - compact RoPE tables (cos|sin [128,512] each + idt) instead of
    per-head-repeated ones: 294KB instead of 544KB on the critical
    input DMA path; products become per-(x,h) [128,256] DVE ops.
  - RoPE runs ONLY on DVE: a GpSimd tensor op concurrent with DVE ops
    slowed DVE ~3x via SBUF port contention in the baseline trace.
  - PE warm-up spam uses REGULAR matmuls (transpose-mode PE work does
    not count as busy for the HAM clock gate, so the old transpose spam
    left phase 2/3 at 1.2 GHz).
  - chunk transposes land 4-per-PSUM-bank so one wide ACT copy
    evacuates 4 chunks (4 copies instead of 16).
  - phase 3 follows phase 2 immediately on a warm PE; output blocks
    cast on DVE and stream out on alternating HWDGE rings.
"""

from contextlib import ExitStack

import numpy as np

import concourse.bass as bass
import concourse.mybir as mybir
import concourse.tile as tile
from concourse.bass_utils import run_bass_kernel_spmd
from concourse.vector_clock import ScopedClock

H, T, D = 16, 2048, 64
N_CORES = 8
HPC = H // N_CORES  # heads per core
P = 128
NT = T // P  # 16 t-chunks per head
HD = D // 2
NTAB = 2 * NT * HD + P  # cos [r,c,kh]=512 | sin 512 | idt 128
F32 = mybir.dt.float32
BF16 = mybir.dt.bfloat16
N_WARM = 26  # dep-free bf16 matmuls: spin HAM to 2.4 GHz before phase 2


def _rope_tables():
    inv_freq = 1.0 / (10000.0 ** (np.arange(0, D, 2, dtype=np.float32) / D))
    t = np.arange(T, dtype=np.float32)
    freqs = np.outer(t, inv_freq).astype(np.float32)  # [T, D/2]
    return np.cos(freqs).astype(np.float32), np.sin(freqs).astype(np.float32)


class _SlimTileContext(tile.TileContext):
    """TileContext whose kernel tail uses per-engine drains + a
    sequencer-level (sem-only) barrier instead of the full EVSEM
    butterfly."""

    def _drain_and_barrier(self, tick_clock, wait_clock):
        nc = self.nc
        drain_inst = nc.sync.drain()
        wait_clock.add_sem_waits(
            drain_inst.ins, ScopedClock({None: tick_clock.global_clock})
        )
        for eng in nc.engines.values():
            if eng.engine != mybir.EngineType.SP:
                eng.drain(fusable=False)
        nc.all_engine_barrier(sem_only=True)
        popped = nc._tile_sem_poison_stack.pop()
        assert popped is self._sem_poison
        nc.clear_and_free_semaphores(list(self.sems.allocated().values()))
        nc.all_engine_barrier(sem_only=True)


def _build_nc():
    nc = bass.Bass()
    TAB = nc.declare_dram_parameter("TAB", [P, NTAB], BF16, isOutput=False)
    # q per range r: [x(rot-half), h, c(8), kh(32)]
    QA = nc.declare_dram_parameter("QA", [P, 2 * HPC * 8 * HD], BF16, isOutput=False)
    QB = nc.declare_dram_parameter("QB", [P, 2 * HPC * 8 * HD], BF16, isOutput=False)
    VA = nc.declare_dram_parameter("VA", [P, 8 * HPC * D], BF16, isOutput=False)
    VB = nc.declare_dram_parameter("VB", [P, 8 * HPC * D], BF16, isOutput=False)
    OUT = nc.declare_dram_parameter("OUT", [P, T], BF16, isOutput=True)

    with _SlimTileContext(nc) as tc, ExitStack() as ctx:
        singles = ctx.enter_context(tc.tile_pool(name="singles", bufs=1))
        ps_warm = ctx.enter_context(tc.tile_pool(name="ps_warm", bufs=1, space="PSUM"))
        ps_s = ctx.enter_context(tc.tile_pool(name="ps_s", bufs=1, space="PSUM"))
        ps_tp = ctx.enter_context(tc.tile_pool(name="ps_tp", bufs=1, space="PSUM"))
        ps_o = ctx.enter_context(tc.tile_pool(name="ps_o", bufs=2, space="PSUM"))

        # --- early, dependency-free work -------------------------------
        spam_src = singles.tile([P, P], BF16)
        s2d = singles.tile([P, P], BF16)
        nc.gpsimd.memset(spam_src[:, 0:2], 0.0)
        nc.gpsimd.memset(s2d, 0.0)  # off-diagonal stays 0 for phase 3

        # Regular (non-transpose) garbage matmuls: start right after the
        # preamble, keep the PE busy while input DMAs land, and DO count
        # as PE-busy for the HAM clock gate.
        for _ in range(N_WARM):
            warm = ps_warm.tile([P, P], F32, tag="w")
            nc.tensor.matmul(warm, lhsT=spam_src, rhs=spam_src, start=True, stop=True)

        # --- input DMAs ------------------------------------------------
        tab_sb = singles.tile([P, NTAB], BF16)
        q_sb = singles.tile([P, 2, 2, HPC, 8, HD], BF16)  # [r, x, h, c, kh]
        v_sb = singles.tile([P, NT, HPC, D], BF16)

        nc.sync.dma_start(
            out=q_sb[:, 0],
            in_=QA[:].rearrange("p (x h c k) -> p x h c k", x=2, h=HPC, c=8),
        )
        nc.scalar.dma_start(out=tab_sb, in_=TAB[:])
        nc.sync.dma_start(
            out=q_sb[:, 1],
            in_=QB[:].rearrange("p (x h c k) -> p x h c k", x=2, h=HPC, c=8),
        )
        nc.scalar.dma_start(
            out=v_sb[:, 0:8],
            in_=VA[:].rearrange("p (c h d) -> p c h d", c=8, h=HPC),
        )
        nc.sync.dma_start(
            out=v_sb[:, 8:16],
            in_=VB[:].rearrange("p (c h d) -> p c h d", c=8, h=HPC),
        )

        cos_t = tab_sb[:, 0 : NT * HD].rearrange("p (r ck) -> p r ck", r=2)
        sin_t = tab_sb[:, NT * HD : 2 * NT * HD].rearrange("p (r ck) -> p r ck", r=2)
        idt = tab_sb[:, 2 * NT * HD :]

        # idt copy doubles as DVE's TAB-semaphore absorber.
        idt_r = singles.tile([P, P], BF16)
        nc.vector.tensor_copy(out=idt_r, in_=idt)

        # --- tiles -----------------------------------------------------
        # products per range: [x, h, c, kh]
        cp = singles.tile([P, 2, HPC, 8, HD], BF16)
        sp = singles.tile([P, 2, HPC, 8, HD], BF16)
        qr_r = singles.tile([P, NT, HPC, 2, HD], BF16)  # [c16, h, x, kh]
        qrt_sb = singles.tile([P, NT * P], BF16)
        outT_sb = singles.tile([P, T], BF16)

        s2_ps = ps_s.tile([P, P], F32)
        tp_g = [
            ps_tp.tile([P, 4 * P], F32, tag=f"tp{g}", name=f"tp{g}") for g in range(4)
        ]

        shp = dict(h=HPC, c=8)
        for r in range(2):
            r0 = r * 8
            cs = slice(r0, r0 + 8)
            # RoPE products, all on DVE, per (x, h) so the compact table
            # slice [p, (c,kh)] lines up with contiguous q slices.
            for h in range(HPC):
                nc.vector.tensor_mul(cp[:, 0, h], q_sb[:, r, 0, h], cos_t[:, r])
                nc.vector.tensor_mul(sp[:, 1, h], q_sb[:, r, 1, h], sin_t[:, r])
            # qr_lo = q_lo*cos - q_hi*sin
            nc.vector.tensor_sub(
                qr_r[:, cs, :, 0, :].rearrange("p c h k -> p h c k"),
                cp[:, 0].rearrange("p h c k -> p h c k"),
                sp[:, 1].rearrange("p h c k -> p h c k"),
            )
            for h in range(HPC):
                nc.vector.tensor_mul(cp[:, 1, h], q_sb[:, r, 1, h], cos_t[:, r])
                nc.vector.tensor_mul(sp[:, 0, h], q_sb[:, r, 0, h], sin_t[:, r])
            # qr_hi = q_hi*cos + q_lo*sin
            nc.vector.tensor_add(
                qr_r[:, cs, :, 1, :].rearrange("p c h k -> p h c k"),
                cp[:, 1].rearrange("p h c k -> p h c k"),
                sp[:, 0].rearrange("p h c k -> p h c k"),
            )

            for c in range(r0, r0 + 8):
                qr2 = qr_r[:, c].rearrange("p h x k -> p (h x k)")
                v2 = v_sb[:, c].rearrange("p h d -> p (h d)")
                nc.tensor.matmul(
                    s2_ps, lhsT=qr2, rhs=v2, start=(c == 0), stop=(c == NT - 1)
                )
                g, j = divmod(c, 4)
                nc.tensor.matmul(
                    tp_g[g][:, j * P : (j + 1) * P],
                    lhsT=qr2,
                    rhs=idt_r,
                    start=True,
                    stop=True,
                )
                if j == 3:
                    # one wide ACT copy evacuates 4 transposed chunks
                    nc.scalar.copy(
                        out=qrt_sb[:, g * 4 * P : (g + 1) * 4 * P], in_=tp_g[g]
                    )

        # Diagonal S_h blocks into the (pre-zeroed) phase-3 operand.
        nc.vector.tensor_copy(out=s2d[:D, :D], in_=s2_ps[:D, :D])
        nc.vector.tensor_copy(out=s2d[D:, D:], in_=s2_ps[D:, D:])

        # outT blocks: blockdiag(S)^T @ QRT serves both heads at once.
        for g in range(4):
            o_ps = ps_o.tile([P, 512], F32, tag="o")
            blk = slice(g * 512, (g + 1) * 512)
            nc.tensor.matmul(
                o_ps, lhsT=s2d, rhs=qrt_sb[:, blk], start=True, stop=True
            )
            nc.vector.tensor_copy(out=outT_sb[:, blk], in_=o_ps)
            eng = nc.sync if g % 2 == 0 else nc.scalar
            eng.dma_start(out=OUT[:, blk], in_=outT_sb[:, blk])

    _split_multi_waits(nc)
    return nc


def _split_multi_waits(nc):
    """This compiler build rejects instructions carrying more than one
    sync-wait command; split extras into single-wait NoOps placed
    immediately before on the same engine."""
    n = 0
    for f in nc.m.functions:
        for blk in f.blocks:
            new_insts = []
            for inst in blk.instructions:
                si = inst.sync_info
                waits = list(si.on_wait) if si else []
                if len(waits) > 1:
                    for w in waits[:-1]:
                        nop = mybir.InstNoOp(name=f"W-split-{n}", ins=[], outs=[])
                        n += 1
                        nop.engine = inst.engine
                        nop.sync_info = mybir.SyncInfo(on_wait=[w], on_update=[])
                        new_insts.append(nop)
                    inst.sync_info = mybir.SyncInfo(
                        on_wait=[waits[-1]], on_update=list(si.on_update)
                    )
                new_insts.append(inst)
            blk.instructions = new_insts


_NC_CACHE = None


def _get_nc():
    global _NC_CACHE
    if _NC_CACHE is None:
        _NC_CACHE = _build_nc()
    return _NC_CACHE


def _pack_inputs(Qs, Vs, cos32, sin32, idt):
    import ml_dtypes

    bf16 = ml_dtypes.bfloat16

    # [T, X] -> [P, NT, X] with t = p*NT + c
    def r(x):
        return x.reshape(P, NT, -1)

    # compact tables: cos[p, r, c, kh], sin[p, r, c, kh], idt
    ce = r(cos32).reshape(P, 2, 8, HD)
    se = r(sin32).reshape(P, 2, 8, HD)
    tab = np.concatenate(
        [ce.reshape(P, -1), se.reshape(P, -1), idt], axis=1
    ).astype(bf16)
    tab = np.ascontiguousarray(tab)

    in_maps = []
    for core in range(N_CORES):
        h0 = core * HPC
        # q[p, r, x, h, c, kh], v[p, c, h, d]
        q = np.empty((P, 2, 2, HPC, 8, HD), np.float32)
        v = np.empty((P, NT, HPC, D), np.float32)
        for h in range(HPC):
            qh = r(Qs[h0 + h]).reshape(P, 2, 8, D)  # [p, r, c, d]
            q[:, :, 0, h] = qh[:, :, :, :HD]
            q[:, :, 1, h] = qh[:, :, :, HD:]
            v[:, :, h] = r(Vs[h0 + h])
        in_maps.append(
            {
                "TAB": tab,
                "QA": np.ascontiguousarray(q[:, 0].reshape(P, -1).astype(bf16)),
                "QB": np.ascontiguousarray(q[:, 1].reshape(P, -1).astype(bf16)),
                "VA": np.ascontiguousarray(v[:, 0:8].reshape(P, -1).astype(bf16)),
                "VB": np.ascontiguousarray(v[:, 8:16].reshape(P, -1).astype(bf16)),
            }
        )
    return in_maps


def _unpack_out(o):
    # o: [P, T] = outT; rows h*64+j, cols c-major: col = c*128 + f, t = f*16+c
    a = o.reshape(HPC, D, NT, P)  # [h, j, c, f]
    return a.transpose(0, 3, 2, 1).reshape(HPC, T, D)  # [h, t=f*16+c, j]


def run_inner(Q, K, V, trace=False):
    del K  # the module sets KR = QR; K is unused
    Qs = np.asarray(Q, dtype=np.float32)[0]  # [H, T, D]
    Vs = np.asarray(V, dtype=np.float32)[0]
    cos32, sin32 = _rope_tables()
    idt = np.eye(P, dtype=np.float32)
    nc = _get_nc()
    in_maps = _pack_inputs(Qs, Vs, cos32, sin32, idt)
    res = run_bass_kernel_spmd(nc, in_maps, list(range(N_CORES)), trace=trace)
    outs = [_unpack_out(np.asarray(res.results[i]["OUT"])) for i in range(N_CORES)]
    out = np.concatenate(outs, axis=0)[None]  # [1, H, T, D]
    return out.astype(np.float32), res


def kernel(Q, K, V):
    out, _ = run_inner(Q, K, V, trace=False)
    return out


# revision 14
# speedup vs baseline: 1.0982x; 1.0023x over previous
"""Trainium2 Bass kernel for nn_LinearAttention (RoPE(Q) @ RoPE(Q)^T @ V).

Algebraic core: no softmax, so out = (QR@QR^T)@V == QR@(QR^T@V) with a
64x64 per-head intermediate. 16 heads / 8 cores = 2 heads per core; the
two heads ride the two 64-wide lanes of the 128x128 PE array.

Layout: t = p*16 + (r*8 + c) (p = SBUF partition, r = range 0/1,
c = chunk-in-range); the host packs/unpacks with this permutation.

v3 schedule:
  - cos (+identity) and sin tables travel on DIFFERENT HWDGE rings and
    their two DMA instructions are hoisted (BIR surgery) before the
    bass-init all-engine barrier, so table data is in flight during the
    last ~us of the fixed preamble. Q lands right behind the tables.
  - RoPE runs ONLY on DVE (GpSimd concurrent with DVE costs DVE ~3x via
    SBUF port contention). Products use an h-broadcast (stride-0) table
    AP; qr is stored [r, x, h, c, kh] so the combines write contiguous
    and the per-chunk matmul lhsT does the (h, x, kh) reorder instead.
  - PE warm-up spam alternates between two PSUM banks (same-bank
    back-to-back matmuls serialize on the fill/drain port) and uses
    regular matmuls (transpose-mode doesn't count for the HAM gate).
  - chunk transposes land 4-per-PSUM-bank; one wide ACT copy evacuates
    each group.
  - phase-3 output blocks: casts alternate DVE/ACT, out-DMAs go on the
    opposite ring, so the block pipeline has no serial engine.
"""

from contextlib import ExitStack

import numpy as np

import concourse.bass as bass
import concourse.mybir as mybir
import concourse.tile as tile
from concourse.bass_utils import run_bass_kernel_spmd
from concourse.vector_clock import ScopedClock

H, T, D = 16, 2048, 64
N_CORES = 8
HPC = H // N_CORES  # heads per core
P = 128
NT = T // P  # 16 t-chunks per head
HD = D // 2
NTABC = NT * HD + P  # cos [r,c,kh]=512 | idt 128
NTABS = NT * HD  # sin 512
F32 = mybir.dt.float32
BF16 = mybir.dt.bfloat16
N_WARM = 34  # dep-free bf16 matmuls: spin HAM to 2.4 GHz before phase 2


def _rope_tables():
    inv_freq = 1.0 / (10000.0 ** (np.arange(0, D, 2, dtype=np.float32) / D))
    t = np.arange(T, dtype=np.float32)
    freqs = np.outer(t, inv_freq).astype(np.float32)  # [T, D/2]
    return np.cos(freqs).astype(np.float32), np.sin(freqs).astype(np.float32)


class _SlimTileContext(tile.TileContext):
    """TileContext whose kernel tail uses per-engine drains + a
    sequencer-level (sem-only) barrier instead of the full EVSEM
    butterfly."""

    def _drain_and_barrier(self, tick_clock, wait_clock):
        nc = self.nc
        drain_inst = nc.sync.drain()
        wait_clock.add_sem_waits(
            drain_inst.ins, ScopedClock({None: tick_clock.global_clock})
        )
        for eng in nc.engines.values():
            if eng.engine != mybir.EngineType.SP:
                eng.drain(fusable=False)
        nc.all_engine_barrier(sem_only=True)
        popped = nc._tile_sem_poison_stack.pop()
        assert popped is self._sem_poison
        nc.clear_and_free_semaphores(list(self.sems.allocated().values()))
        nc.all_engine_barrier(sem_only=True)


def _build_nc():
    nc = bass.Bass()
    TABC = nc.declare_dram_parameter("TABC", [P, NTABC], BF16, isOutput=False)
    TABS = nc.declare_dram_parameter("TABS", [P, NTABS], BF16, isOutput=False)
    # q per range r: [x(rot-half), h, c(8), kh(32)]
    QA = nc.declare_dram_parameter("QA", [P, 2 * HPC * 8 * HD], BF16, isOutput=False)
    QB = nc.declare_dram_parameter("QB", [P, 2 * HPC * 8 * HD], BF16, isOutput=False)
    VA = nc.declare_dram_parameter("VA", [P, 8 * HPC * D], BF16, isOutput=False)
    VB = nc.declare_dram_parameter("VB", [P, 8 * HPC * D], BF16, isOutput=False)
    OUT = nc.declare_dram_parameter("OUT", [P, T], BF16, isOutput=True)

    hoist_names = []

    with _SlimTileContext(nc) as tc, ExitStack() as ctx:
        singles = ctx.enter_context(tc.tile_pool(name="singles", bufs=1))
        ps_s = ctx.enter_context(tc.tile_pool(name="ps_s", bufs=1, space="PSUM"))
        ps_tp = ctx.enter_context(tc.tile_pool(name="ps_tp", bufs=1, space="PSUM"))
        ps_o = ctx.enter_context(tc.tile_pool(name="ps_o", bufs=2, space="PSUM"))

        # --- input DMAs (table DMAs get hoisted pre-barrier) -----------
        tabc_sb = singles.tile([P, NTABC], BF16)
        tabs_sb = singles.tile([P, NTABS], BF16)
        q_sb = singles.tile([P, 2, 2, HPC, 8, HD], BF16)  # [r, x, h, c, kh]
        v_sb = singles.tile([P, NT, HPC, D], BF16)

        i1 = nc.sync.dma_start(out=tabc_sb, in_=TABC[:])
        i2 = nc.scalar.dma_start(out=tabs_sb, in_=TABS[:])
        hoist_names += [i1.ins.name, i2.ins.name]
        nc.sync.dma_start(
            out=q_sb[:, 0],
            in_=QA[:].rearrange("p (x h c k) -> p x h c k", x=2, h=HPC, c=8),
        )
        nc.scalar.dma_start(
            out=v_sb[:, 0:8],
            in_=VA[:].rearrange("p (c h d) -> p c h d", c=8, h=HPC),
        )
        nc.sync.dma_start(
            out=q_sb[:, 1],
            in_=QB[:].rearrange("p (x h c k) -> p x h c k", x=2, h=HPC, c=8),
        )
        nc.scalar.dma_start(
            out=v_sb[:, 8:16],
            in_=VB[:].rearrange("p (c h d) -> p c h d", c=8, h=HPC),
        )

        # --- early, dependency-free work -------------------------------
        spam_src = singles.tile([P, P], BF16)
        s2d = singles.tile([P, P], BF16)
        nc.gpsimd.memset(spam_src[:, 0:2], 0.0)
        nc.gpsimd.memset(s2d, 0.0)  # off-diagonal stays 0 for phase 3

        # Regular matmuls rotating across the four (still-unused) tp PSUM
        # banks: back-to-back issue (same-bank MMs serialize on the bank
        # port), counts as PE-busy for HAM, keeps the PE warm until
        # phase 2 is ready. The real transposes later overwrite the spam.

        cos_t = tabc_sb[:, 0 : NT * HD].rearrange("p (r a c k) -> p r a c k", r=2, a=1, c=8)
        idt = tabc_sb[:, NT * HD :]
        sin_t = tabs_sb.rearrange("p (r a c k) -> p r a c k", r=2, a=1, c=8)

        # idt copy doubles as DVE's TABC-semaphore absorber.
        idt_r = singles.tile([P, P], BF16)
        nc.vector.tensor_copy(out=idt_r, in_=idt)

        # --- tiles -----------------------------------------------------
        cp = singles.tile([P, 2, HPC, 8, HD], BF16)  # [x, h, c, kh]
        sp = singles.tile([P, 2, HPC, 8, HD], BF16)
        qr_r = singles.tile([P, NT, 2, HPC, HD], BF16)  # [c16, x, h, kh]
        qrt_sb = singles.tile([P, NT * P], BF16)
        outT_sb = singles.tile([P, T], BF16)

        s2_ps = ps_s.tile([P, P], F32)
        tp_g = [
            ps_tp.tile([P, 4 * P], F32, tag=f"tp{g}", name=f"tp{g}") for g in range(4)
        ]
        for i in range(N_WARM):
            nc.tensor.matmul(
                tp_g[i % 4][:, 0:P],
                lhsT=spam_src,
                rhs=spam_src,
                start=True,
                stop=True,
            )

        bshape = [P, HPC, 8, HD]
        for r in range(2):
            cosb = cos_t[:, r].to_broadcast(bshape)
            sinb = sin_t[:, r].to_broadcast(bshape)
            # RoPE products on DVE with h-broadcast tables; contiguous IO.
            nc.vector.tensor_mul(cp[:, 0], q_sb[:, r, 0], cosb)
            nc.vector.tensor_mul(sp[:, 1], q_sb[:, r, 1], sinb)
            cs = slice(r * 8, r * 8 + 8)
            # qr_lo = q_lo*cos - q_hi*sin  (chunk-major scatter write)
            nc.vector.tensor_sub(
                qr_r[:, cs, 0].rearrange("p c h k -> p h c k"), cp[:, 0], sp[:, 1]
            )
            nc.vector.tensor_mul(cp[:, 1], q_sb[:, r, 1], cosb)
            nc.vector.tensor_mul(sp[:, 0], q_sb[:, r, 0], sinb)
            # qr_hi = q_hi*cos + q_lo*sin
            nc.vector.tensor_add(
                qr_r[:, cs, 1].rearrange("p c h k -> p h c k"), cp[:, 1], sp[:, 0]
            )

            for j in range(8):
                c = r * 8 + j
                # rows in (h, x, kh) lane order via AP permutation
                # rows in (x, h, kh) order; the chunk slice is fully
                # contiguous, which the weights-AP verifier requires.
                # The head-selection in s2d below matches this row order.
                qr2 = qr_r[:, c].rearrange("p x h k -> p (x h k)")
                v2 = v_sb[:, c].rearrange("p h d -> p (h d)")
                nc.tensor.matmul(
                    s2_ps, lhsT=qr2, rhs=v2, start=(c == 0), stop=(c == NT - 1)
                )
                g, jj = divmod(c, 4)
                nc.tensor.matmul(
                    tp_g[g][:, jj * P : (jj + 1) * P],
                    lhsT=qr2,
                    rhs=idt_r,
                    start=True,
                    stop=True,
                )
                if jj == 3:
                    # one wide ACT copy evacuates 4 transposed chunks
                    nc.scalar.copy(
                        out=qrt_sb[:, g * 4 * P : (g + 1) * 4 * P], in_=tp_g[g]
                    )

        # Head-diagonal blocks of S2 into the (pre-zeroed) phase-3
        # operand. Partition rows are (x, h, kh): head h owns rows
        # {x*64 + h*32 .. +32}; its columns are h*64..h*64+64.
        nc.vector.tensor_copy(out=s2d[0:32, 0:D], in_=s2_ps[0:32, 0:D])
        nc.vector.tensor_copy(out=s2d[32:64, D:], in_=s2_ps[32:64, D:])
        nc.vector.tensor_copy(out=s2d[64:96, 0:D], in_=s2_ps[64:96, 0:D])
        nc.vector.tensor_copy(out=s2d[96:128, D:], in_=s2_ps[96:128, D:])

        # outT blocks: blockdiag(S)^T @ QRT serves both heads at once.
        # Casts alternate DVE/ACT; each block's out-DMA rides the ring
        # whose engine did NOT do the cast.
        for g in range(4):
            o_ps = ps_o.tile([P, 512], F32, tag="o")
            blk = slice(g * 512, (g + 1) * 512)
            nc.tensor.matmul(
                o_ps, lhsT=s2d, rhs=qrt_sb[:, blk], start=True, stop=True
            )
            if g % 2 == 0:
                nc.vector.tensor_copy(out=outT_sb[:, blk], in_=o_ps)
                nc.scalar.dma_start(out=OUT[:, blk], in_=outT_sb[:, blk])
            else:
                nc.scalar.copy(out=outT_sb[:, blk], in_=o_ps)
                nc.sync.dma_start(out=OUT[:, blk], in_=outT_sb[:, blk])

    _split_multi_waits(nc)
    _hoist_input_dmas(nc, hoist_names)
    return nc


def _split_multi_waits(nc):
    """This compiler build rejects instructions carrying more than one
    sync-wait command; split extras into single-wait NoOps placed
    immediately before on the same engine."""
    n = 0
    for f in nc.m.functions:
        for blk in f.blocks:
            new_insts = []
            for inst in blk.instructions:
                si = inst.sync_info
                waits = list(si.on_wait) if si else []
                if len(waits) > 1:
                    for w in waits[:-1]:
                        nop = mybir.InstNoOp(name=f"W-split-{n}", ins=[], outs=[])
                        n += 1
                        nop.engine = inst.engine
                        nop.sync_info = mybir.SyncInfo(on_wait=[w], on_update=[])
                        new_insts.append(nop)
                    inst.sync_info = mybir.SyncInfo(
                        on_wait=[waits[-1]], on_update=list(si.on_update)
                    )
                new_insts.append(inst)
            blk.instructions = new_insts


def _hoist_input_dmas(nc, names):
    """Move the (dependency-free) table DMA issues from the kernel body
    to just before each engine's entry-barrier instruction in `main`, so
    the transfers are in flight during the tail of the fixed preamble.
    The DMA semaphores are runtime-zeroed before the NEFF starts and the
    consumers wait on absolute sem values, so only issue order matters;
    per-engine program order is preserved."""
    names = set(names)
    f = nc.m.functions[0]
    blocks = {b.name: b for b in f.blocks}
    main = blocks["main"]
    moved = []
    for b in f.blocks:
        if b.name == "main":
            continue
        keep = []
        for inst in b.instructions:
            if inst.name in names:
                si = inst.sync_info
                assert not (si and si.on_wait), f"hoisted DMA {inst.name} has waits"
                moved.append(inst)
            else:
                keep.append(inst)
        if len(keep) != len(b.instructions):
            b.instructions = keep
    assert len(moved) == len(names), (len(moved), names)
    new_main = []
    barrier_seen = set()
    for inst in main.instructions:
        if inst.name.startswith("barrier_") and inst.engine not in barrier_seen:
            barrier_seen.add(inst.engine)
            for m in moved:
                if m.engine == inst.engine:
                    new_main.append(m)
        new_main.append(inst)
    main.instructions = new_main


_NC_CACHE = None


def _get_nc():
    global _NC_CACHE
    if _NC_CACHE is None:
        _NC_CACHE = _build_nc()
    return _NC_CACHE


def _pack_inputs(Qs, Vs, cos32, sin32, idt):
    import ml_dtypes

    bf16 = ml_dtypes.bfloat16

    # [T, X] -> [P, NT, X] with t = p*NT + c
    def r(x):
        return x.reshape(P, NT, -1)

    ce = r(cos32).reshape(P, 2, 8, HD)  # [p, r, c, kh]
    se = r(sin32).reshape(P, 2, 8, HD)
    tabc = np.ascontiguousarray(
        np.concatenate([ce.reshape(P, -1), idt], axis=1).astype(bf16)
    )
    tabs = np.ascontiguousarray(se.reshape(P, -1).astype(bf16))

    in_maps = []
    for core in range(N_CORES):
        h0 = core * HPC
        # q[p, r, x, h, c, kh], v[p, c, h, d]
        q = np.empty((P, 2, 2, HPC, 8, HD), np.float32)
        v = np.empty((P, NT, HPC, D), np.float32)
        for h in range(HPC):
            qh = r(Qs[h0 + h]).reshape(P, 2, 8, D)  # [p, r, c, d]
            q[:, :, 0, h] = qh[:, :, :, :HD]
            q[:, :, 1, h] = qh[:, :, :, HD:]
            v[:, :, h] = r(Vs[h0 + h])
        in_maps.append(
            {
                "TABC": tabc,
                "TABS": tabs,
                "QA": np.ascontiguousarray(q[:, 0].reshape(P, -1).astype(bf16)),
                "QB": np.ascontiguousarray(q[:, 1].reshape(P, -1).astype(bf16)),
                "VA": np.ascontiguousarray(v[:, 0:8].reshape(P, -1).astype(bf16)),
                "VB": np.ascontiguousarray(v[:, 8:16].reshape(P, -1).astype(bf16)),
            }
        )
    return in_maps


def _unpack_out(o):
    # o: [P, T] = outT; rows h*64+j, cols c-major: col = c*128 + f, t = f*16+c
    a = o.reshape(HPC, D, NT, P)  # [h, j, c, f]
    return a.transpose(0, 3, 2, 1).reshape(HPC, T, D)  # [h, t=f*16+c, j]


def run_inner(Q, K, V, trace=False):
    del K  # the module sets KR = QR; K is unused
    Qs = np.asarray(Q, dtype=np.float32)[0]  # [H, T, D]
    Vs = np.asarray(V, dtype=np.float32)[0]
    cos32, sin32 = _rope_tables()
    idt = np.eye(P, dtype=np.float32)
    nc = _get_nc()
    in_maps = _pack_inputs(Qs, Vs, cos32, sin32, idt)
    res = run_bass_kernel_spmd(nc, in_maps, list(range(N_CORES)), trace=trace)
    outs = [_unpack_out(np.asarray(res.results[i]["OUT"])) for i in range(N_CORES)]
    out = np.concatenate(outs, axis=0)[None]  # [1, H, T, D]
    return out.astype(np.float32), res


def kernel(Q, K, V):
    out, _ = run_inner(Q, K, V, trace=False)
    return out
